# revision 1
# baseline (speedup 1.0000x reference)
"""CrossAttend Trainium2 kernel: 8-way data-parallel over batch.

Full inputs arrive here; we shard batch B=16 across 8 NeuronCores
(2 batch elements per core), replicate the 512x512 projection weights,
run one SPMD Bass/Tile kernel, and concatenate the per-core outputs.

Math notes (validated against the fp32 reference):
  - bk drops out entirely: it shifts every sim row by a constant per q,
    and softmax over k is shift-invariant.
  - qpk := qp @ Wk is shared by both attentions:
        sim  = qpk @ qp.T   (+ per-q const),   sim2 = qpk @ opp.T (+ const)
  - softmax is computed without max-subtraction (logits are O(5), exp is
    safe in fp32); the self-attention diagonal is zeroed after exp.
  - rowsums come from an extra N=1 matmul against a ones column that
    reuses the PE-resident P^T weights.
All matmuls run as float32r (full-rate PE mode); transposes as fp32.

On-chip layouts per batch element:
  qT, qpT, qpkT, oppT : [128, 4, 1024]  (h on partitions)
  v, opp_v            : [128, 8, 512]   (l on partitions)
  PexpT               : [128, 8, 1024]  (k on partitions, q free)
"""

import contextlib
import math

import numpy as np

import concourse.bass as bass
import concourse.mybir as mybir
import concourse.tile as tile
from concourse import bacc
from concourse.bass_utils import run_bass_kernel_spmd
from concourse.masks import make_identity

F32 = mybir.dt.float32
F32R = mybir.dt.float32r

B = 16
H = 512
L = 1024
P = 128
NCORES = 8
BPC = B // NCORES   # batch elements per core
HT = H // P         # 4 h-tiles
LT = L // P         # 8 l-tiles
QC = L // 512       # 2 q-chunks of 512
SCALE = 1.0 / math.sqrt(H)


def _r(ap):
    return ap.bitcast(F32R)


def _build_core_kernel(ctx, tc, ins, outs):
    nc = tc.nc
    AF = mybir.ActivationFunctionType

    q_d = ins["q"]          # [BPC, L, H]
    opp_d = ins["opp"]      # [BPC, L, H]
    self_d = outs["self_out"]
    oout_d = outs["opp_out"]

    wpool = ctx.enter_context(tc.tile_pool(name="w", bufs=1))
    stage = ctx.enter_context(tc.tile_pool(name="stage", bufs=8))
    big = ctx.enter_context(tc.tile_pool(name="big", bufs=4))
    vpool = ctx.enter_context(tc.tile_pool(name="v", bufs=2))
    ppool = ctx.enter_context(tc.tile_pool(name="P", bufs=1))
    opool = ctx.enter_context(tc.tile_pool(name="o", bufs=4))
    rpool = ctx.enter_context(tc.tile_pool(name="r", bufs=4))
    ps_mm = ctx.enter_context(tc.tile_pool(name="psmm", bufs=4, space="PSUM"))
    ps_tr = ctx.enter_context(tc.tile_pool(name="pstr", bufs=2, space="PSUM"))
    ps_rs = ctx.enter_context(tc.tile_pool(name="psrs", bufs=2, space="PSUM"))

    # --- constants (per-core replicated) ---
    wq = wpool.tile([P, HT, H], F32R, tag="wq")
    nc.gpsimd.dma_start(wq[:], ins["WqT"].bitcast(F32R).rearrange("(ko ki) m -> ki ko m", ki=P))
    wk = wpool.tile([P, HT, H], F32R, tag="wk")
    nc.gpsimd.dma_start(wk[:], ins["Wk"].bitcast(F32R).rearrange("(ko ki) m -> ki ko m", ki=P))
    wv = wpool.tile([P, HT, H], F32R, tag="wv")
    nc.gpsimd.dma_start(wv[:], ins["WvT"].bitcast(F32R).rearrange("(ko ki) m -> ki ko m", ki=P))
    bq = wpool.tile([P, HT], F32, tag="bq")
    nc.gpsimd.dma_start(bq[:], ins["bq_p"][:])
    bvb = wpool.tile([P, H], F32, tag="bvb")
    nc.gpsimd.dma_start(bvb[:], ins["bv_b"][:])
    ident = wpool.tile([P, P], F32, tag="ident")
    make_identity(nc, ident[:])
    ones = wpool.tile([P, 4], F32R, tag="ones")
    nc.gpsimd.dma_start(ones[:], ins["ones_p"].bitcast(F32R)[:])

    def load_transposed(src_d, b):
        """DMA [L, H] natural, PE-transpose into [128, HT, L]."""
        xT = big.tile([P, HT, L], F32R, tag="big")
        for lt in range(LT):
            st = stage.tile([P, H], F32, tag="stage")
            nc.sync.dma_start(st[:], src_d[b, P * lt:P * (lt + 1), :])
            for ht in range(HT):
                pst = ps_tr.tile([P, P], F32, tag="pstr")
                nc.tensor.transpose(pst[:], st[:, P * ht:P * (ht + 1)], ident[:])
                nc.vector.tensor_copy(xT[:, ht, P * lt:P * (lt + 1)], pst[:])
        return xT

    def proj_T(src_T, w, bias=None):
        """dst[h_out-part, l] = sum_hin w[hin, hout-tile].T @ src_T[hin, l]."""
        dst = big.tile([P, HT, L], F32R, tag="big")
        for ht in range(HT):
            for qc in range(QC):
                ps = ps_mm.tile([P, 512], F32, tag="psmm")
                for hc in range(HT):
                    nc.tensor.matmul(
                        ps[:],
                        lhsT=(w[:, hc, P * ht:P * (ht + 1)]),
                        rhs=(src_T[:, hc, 512 * qc:512 * (qc + 1)]),
                        start=(hc == 0),
                        stop=(hc == HT - 1),
                    )
                d = dst[:, ht, 512 * qc:512 * (qc + 1)]
                if bias is not None:
                    nc.scalar.activation(d, ps[:], AF.Identity,
                                         bias=bias[:, ht:ht + 1], scale=1.0)
                else:
                    nc.vector.tensor_copy(d, ps[:])
        return dst

    def proj_nat(src_T, w_rhs, bias_b):
        """dst[l-part, h_out] = src_T[hin, l-tile].T @ w_rhs[hin, hout] + bias."""
        dst = vpool.tile([P, LT, H], F32R, tag="v")
        for lt in range(LT):
            ps = ps_mm.tile([P, 512], F32, tag="psmm")
            for hc in range(HT):
                nc.tensor.matmul(
                    ps[:],
                    lhsT=(src_T[:, hc, P * lt:P * (lt + 1)]),
                    rhs=(w_rhs[:, hc, :]),
                    start=(hc == 0),
                    stop=(hc == HT - 1),
                )
            nc.vector.tensor_tensor(dst[:, lt, :], ps[:], bias_b[:],
                                    mybir.AluOpType.add)
        return dst

    def attn(lhsT_T, qpkT, vv, out_d, b, masked):
        """PexpT[k, q] = exp(scale * lhsT_T.T @ qpkT); out = (P.T @ v)/rowsum."""
        pexp = ppool.tile([P, LT, L], F32R, tag="P")
        for ko in range(LT):
            for qc in range(QC):
                ps = ps_mm.tile([P, 512], F32, tag="psmm")
                for hc in range(HT):
                    nc.tensor.matmul(
                        ps[:],
                        lhsT=(lhsT_T[:, hc, P * ko:P * (ko + 1)]),
                        rhs=(qpkT[:, hc, 512 * qc:512 * (qc + 1)]),
                        start=(hc == 0),
                        stop=(hc == HT - 1),
                    )
                d = pexp[:, ko, 512 * qc:512 * (qc + 1)]
                nc.scalar.activation(d, ps[:], AF.Exp, scale=SCALE)
                if masked and qc == ko // (512 // P):
                    m = ko % (512 // P)
                    nc.gpsimd.affine_select(
                        out=d, in_=d,
                        compare_op=mybir.AluOpType.not_equal,
                        fill=0.0, base=P * m,
                        pattern=[[-1, 512]], channel_multiplier=1,
                    )
        for qo in range(LT):
            pso = ps_mm.tile([P, 512], F32, tag="psmm")
            psr = ps_rs.tile([P, 4], F32, tag="psrs")
            for ko in range(LT):
                nc.tensor.matmul(
                    pso[:], lhsT=(pexp[:, ko, P * qo:P * (qo + 1)]),
                    rhs=(vv[:, ko, :]),
                    start=(ko == 0), stop=(ko == LT - 1),
                )
                nc.tensor.matmul(
                    psr[:], lhsT=(pexp[:, ko, P * qo:P * (qo + 1)]),
                    rhs=(ones[:, 0:4]),
                    start=(ko == 0), stop=(ko == LT - 1),
                )
            rc = rpool.tile([P, 1], F32, tag="r")
            nc.vector.reciprocal(rc[:], psr[:, 0:1])
            ot = opool.tile([P, 512], F32, tag="o")
            nc.scalar.activation(ot[:], pso[:], AF.Copy, scale=rc[:, 0:1])
            nc.sync.dma_start(out_d[b, P * qo:P * (qo + 1), :], ot[:])

    for b in range(BPC):
        qT = load_transposed(q_d, b)
        qpT = proj_T(qT, wq, bias=bq)
        qpkT = proj_T(qpT, wk)
        vv = proj_nat(qpT, wv, bvb)
        oppT = load_transposed(opp_d, b)
        ovv = proj_nat(oppT, wv, bvb)
        attn(qpT, qpkT, vv, self_d, b, masked=True)
        attn(oppT, qpkT, ovv, oout_d, b, masked=False)


_NC_CACHE = None


def _get_module():
    global _NC_CACHE
    if _NC_CACHE is not None:
        return _NC_CACHE
    nc = bacc.Bacc(None, target_bir_lowering=False, debug=False)
    f32 = mybir.dt.float32
    ins = {
        "q": nc.dram_tensor("q", [BPC, L, H], f32, kind="ExternalInput").ap(),
        "opp": nc.dram_tensor("opp", [BPC, L, H], f32, kind="ExternalInput").ap(),
        "WqT": nc.dram_tensor("WqT", [H, H], f32, kind="ExternalInput").ap(),
        "Wk": nc.dram_tensor("Wk", [H, H], f32, kind="ExternalInput").ap(),
        "WvT": nc.dram_tensor("WvT", [H, H], f32, kind="ExternalInput").ap(),
        "bq_p": nc.dram_tensor("bq_p", [P, HT], f32, kind="ExternalInput").ap(),
        "bv_b": nc.dram_tensor("bv_b", [P, H], f32, kind="ExternalInput").ap(),
        "ones_p": nc.dram_tensor("ones_p", [P, 4], f32, kind="ExternalInput").ap(),
    }
    outs = {
        "self_out": nc.dram_tensor("self_out", [BPC, L, H], f32,
                                   kind="ExternalOutput").ap(),
        "opp_out": nc.dram_tensor("opp_out", [BPC, L, H], f32,
                                  kind="ExternalOutput").ap(),
    }
    with tile.TileContext(nc) as tc:
        with contextlib.ExitStack() as ctx:
            _build_core_kernel(ctx, tc, ins, outs)
    nc.compile()
    _NC_CACHE = nc
    return nc


def kernel(q, opp, Wq, bq, Wk, bk, Wv, bv):
    q = np.ascontiguousarray(np.asarray(q, dtype=np.float32))
    opp = np.ascontiguousarray(np.asarray(opp, dtype=np.float32))
    Wq = np.asarray(Wq, dtype=np.float32)
    Wk = np.asarray(Wk, dtype=np.float32)
    Wv = np.asarray(Wv, dtype=np.float32)
    bq = np.asarray(bq, dtype=np.float32)
    bv = np.asarray(bv, dtype=np.float32)
    # bk is mathematically irrelevant (softmax shift-invariance); unused.

    shared = {
        "WqT": np.ascontiguousarray(Wq.T),
        "Wk": np.ascontiguousarray(Wk),
        "WvT": np.ascontiguousarray(Wv.T),
        "bq_p": np.ascontiguousarray(bq.reshape(HT, P).T),
        "bv_b": np.ascontiguousarray(np.tile(bv, (P, 1))),
        "ones_p": np.ones((P, 4), dtype=np.float32),
    }
    in_maps = []
    for c in range(NCORES):
        sl = slice(c * BPC, (c + 1) * BPC)
        in_maps.append({
            "q": np.ascontiguousarray(q[sl]),
            "opp": np.ascontiguousarray(opp[sl]),
            **shared,
        })

    nc = _get_module()
    res = run_bass_kernel_spmd(nc, in_maps, core_ids=list(range(NCORES)))
    self_out = np.concatenate([r["self_out"] for r in res.results], axis=0)
    opp_out = np.concatenate([r["opp_out"] for r in res.results], axis=0)
    return (self_out, opp_out)



# revision 5
# speedup vs baseline: 1.2665x; 1.2665x over previous
"""CrossAttend Trainium2 kernel: 8-way data-parallel over batch.

Full inputs arrive here; we shard batch B=16 across 8 NeuronCores
(2 batch elements per core), replicate the 512x512 projection weights,
run one SPMD Bass/Tile kernel, and concatenate the per-core outputs.

Math notes (validated against the fp32 reference):
  - bk drops out entirely: it shifts every sim row by a constant per q,
    and softmax over k is shift-invariant.
  - qpk := qp @ Wk is shared by both attentions:
        sim  = qpk @ qp.T   (+ per-q const),   sim2 = qpk @ opp.T (+ const)
  - softmax is computed without max-subtraction (logits are O(5), exp is
    safe in fp32); the self-attention diagonal is zeroed after exp.
  - rowsums come from an extra N=4 matmul against a ones column that
    reuses the PE-resident P^T weights.

v2 perf changes (measured on the v1 trace):
  - All matmul operands are bf16 (inputs/weights pre-cast on host).
    bf16 streams at the same 1 cycle/row as fp32r, but stationary
    operands get FWL (4x faster LDWEIGHTS) which hides the weight
    loads that stalled 40% of v1's matmuls (+107ns each).
  - Input transposes moved off the PE entirely: q/opp are uploaded as
    bf16 and loaded pre-transposed via the DMA xbar transpose
    (dma_start_transpose), which fp32 could not use.
  - PSUM accumulation stays fp32; exp() output and P are bf16; the
    numpy-emulated end-to-end error of this mix is ~6e-3 (budget 2e-2).

On-chip layouts per batch element (all bf16 unless noted):
  qT, qpT, qpkT, oppT : [128, 4, 1024]  (h on partitions)
  v, opp_v            : [128, 8, 512]   (l on partitions)
  PexpT               : [128, 8, 1024]  (k on partitions, q free)
"""

import contextlib
import math

import ml_dtypes
import numpy as np

import concourse.bass as bass
import concourse.mybir as mybir
import concourse.tile as tile
from concourse import bacc
from concourse.bass_utils import run_bass_kernel_spmd

F32 = mybir.dt.float32
B16 = mybir.dt.bfloat16

B = 16
H = 512
L = 1024
P = 128
NCORES = 8
BPC = B // NCORES   # batch elements per core
HT = H // P         # 4 h-tiles
LT = L // P         # 8 l-tiles
QC = L // 512       # 2 q-chunks of 512
SCALE = 1.0 / math.sqrt(H)


def _build_core_kernel(ctx, tc, ins, outs):
    nc = tc.nc
    AF = mybir.ActivationFunctionType

    q_d = ins["q"]          # [BPC, L, H] bf16
    opp_d = ins["opp"]      # [BPC, L, H] bf16
    self_d = outs["self_out"]
    oout_d = outs["opp_out"]

    wpool = ctx.enter_context(tc.tile_pool(name="w", bufs=1))
    big = ctx.enter_context(tc.tile_pool(name="big", bufs=6))
    vpool = ctx.enter_context(tc.tile_pool(name="v", bufs=3))
    ppool = ctx.enter_context(tc.tile_pool(name="P", bufs=2))
    opool = ctx.enter_context(tc.tile_pool(name="o", bufs=4))
    rpool = ctx.enter_context(tc.tile_pool(name="r", bufs=4))
    ps_mm = ctx.enter_context(tc.tile_pool(name="psmm", bufs=4, space="PSUM"))
    ps_rs = ctx.enter_context(tc.tile_pool(name="psrs", bufs=2, space="PSUM"))

    # --- constants (per-core replicated) ---
    wq = wpool.tile([P, HT, H], B16, tag="wq")
    nc.gpsimd.dma_start(wq[:], ins["WqT"].rearrange("(ko ki) m -> ki ko m", ki=P))
    wk = wpool.tile([P, HT, H], B16, tag="wk")
    nc.gpsimd.dma_start(wk[:], ins["Wk"].rearrange("(ko ki) m -> ki ko m", ki=P))
    wv = wpool.tile([P, HT, H], B16, tag="wv")
    nc.gpsimd.dma_start(wv[:], ins["WvT"].rearrange("(ko ki) m -> ki ko m", ki=P))
    bq = wpool.tile([P, HT], F32, tag="bq")
    nc.gpsimd.dma_start(bq[:], ins["bq_p"][:])
    bvb = wpool.tile([P, H], F32, tag="bvb")
    nc.gpsimd.dma_start(bvb[:], ins["bv_b"][:])
    ones = wpool.tile([P, 4], B16, tag="ones")
    nc.gpsimd.dma_start(ones[:], ins["ones_p"][:])

    def load_transposed(src_d, b):
        """DMA xbar transpose [L, H] bf16 -> [128, HT, L] (h on partitions).

        All xbar transposes go on one queue (nc.sync): concurrent
        DMA-transposes on two HWDGE queues corrupted the loads."""
        xT = big.tile([P, HT, L], B16, tag="big")
        for ht in range(HT):
            nc.sync.dma_start_transpose(
                xT[:, ht, :], src_d[b, :, P * ht:P * (ht + 1)])
        return xT

    def proj_T(src_T, w, bias=None):
        """dst[h_out-part, l] = sum_hin w[hin, hout-tile].T @ src_T[hin, l]."""
        dst = big.tile([P, HT, L], B16, tag="big")
        for ht in range(HT):
            for qc in range(QC):
                ps = ps_mm.tile([P, 512], F32, tag="psmm")
                for hc in range(HT):
                    nc.tensor.matmul(
                        ps[:],
                        lhsT=(w[:, hc, P * ht:P * (ht + 1)]),
                        rhs=(src_T[:, hc, 512 * qc:512 * (qc + 1)]),
                        start=(hc == 0),
                        stop=(hc == HT - 1),
                    )
                d = dst[:, ht, 512 * qc:512 * (qc + 1)]
                if bias is not None:
                    nc.scalar.activation(d, ps[:], AF.Identity,
                                         bias=bias[:, ht:ht + 1], scale=1.0)
                else:
                    nc.vector.tensor_copy(d, ps[:])
        return dst

    def proj_nat(src_T, w_rhs, bias_b):
        """dst[l-part, h_out] = src_T[hin, l-tile].T @ w_rhs[hin, hout] + bias."""
        dst = vpool.tile([P, LT, H], B16, tag="v")
        for lt in range(LT):
            ps = ps_mm.tile([P, 512], F32, tag="psmm")
            for hc in range(HT):
                nc.tensor.matmul(
                    ps[:],
                    lhsT=(src_T[:, hc, P * lt:P * (lt + 1)]),
                    rhs=(w_rhs[:, hc, :]),
                    start=(hc == 0),
                    stop=(hc == HT - 1),
                )
            nc.vector.tensor_tensor(dst[:, lt, :], ps[:], bias_b[:],
                                    mybir.AluOpType.add)
        return dst

    def attn(lhsT_T, qpkT, vv, out_d, b, masked):
        """PexpT[k, q] = exp(scale * lhsT_T.T @ qpkT); out = (P.T @ v)/rowsum."""
        pexp = ppool.tile([P, LT, L], B16, tag="P")
        for ko in range(LT):
            for qc in range(QC):
                ps = ps_mm.tile([P, 512], F32, tag="psmm")
                for hc in range(HT):
                    nc.tensor.matmul(
                        ps[:],
                        lhsT=(lhsT_T[:, hc, P * ko:P * (ko + 1)]),
                        rhs=(qpkT[:, hc, 512 * qc:512 * (qc + 1)]),
                        start=(hc == 0),
                        stop=(hc == HT - 1),
                    )
                d = pexp[:, ko, 512 * qc:512 * (qc + 1)]
                nc.scalar.activation(d, ps[:], AF.Exp, scale=SCALE)
                if masked and qc == ko // (512 // P):
                    m = ko % (512 // P)
                    nc.gpsimd.affine_select(
                        out=d, in_=d,
                        compare_op=mybir.AluOpType.not_equal,
                        fill=0.0, base=P * m,
                        pattern=[[-1, 512]], channel_multiplier=1,
                    )
        for qo in range(LT):
            pso = ps_mm.tile([P, 512], F32, tag="psmm")
            psr = ps_rs.tile([P, 4], F32, tag="psrs")
            for ko in range(LT):
                nc.tensor.matmul(
                    pso[:], lhsT=(pexp[:, ko, P * qo:P * (qo + 1)]),
                    rhs=(vv[:, ko, :]),
                    start=(ko == 0), stop=(ko == LT - 1),
                )
                nc.tensor.matmul(
                    psr[:], lhsT=(pexp[:, ko, P * qo:P * (qo + 1)]),
                    rhs=(ones[:, 0:4]),
                    start=(ko == 0), stop=(ko == LT - 1),
                )
            rc = rpool.tile([P, 1], F32, tag="r")
            nc.vector.reciprocal(rc[:], psr[:, 0:1])
            ot = opool.tile([P, 512], F32, tag="o")
            nc.scalar.activation(ot[:], pso[:], AF.Copy, scale=rc[:, 0:1])
            nc.scalar.dma_start(out_d[b, P * qo:P * (qo + 1), :], ot[:])

    for b in range(BPC):
        qT = load_transposed(q_d, b)
        qpT = proj_T(qT, wq, bias=bq)
        qpkT = proj_T(qpT, wk)
        vv = proj_nat(qpT, wv, bvb)
        oppT = load_transposed(opp_d, b)
        ovv = proj_nat(oppT, wv, bvb)
        attn(qpT, qpkT, vv, self_d, b, masked=True)
        attn(oppT, qpkT, ovv, oout_d, b, masked=False)


_NC_CACHE = None


def _get_module():
    global _NC_CACHE
    if _NC_CACHE is not None:
        return _NC_CACHE
    nc = bacc.Bacc(None, target_bir_lowering=False, debug=False)
    f32 = mybir.dt.float32
    b16 = mybir.dt.bfloat16
    ins = {
        "q": nc.dram_tensor("q", [BPC, L, H], b16, kind="ExternalInput").ap(),
        "opp": nc.dram_tensor("opp", [BPC, L, H], b16, kind="ExternalInput").ap(),
        "WqT": nc.dram_tensor("WqT", [H, H], b16, kind="ExternalInput").ap(),
        "Wk": nc.dram_tensor("Wk", [H, H], b16, kind="ExternalInput").ap(),
        "WvT": nc.dram_tensor("WvT", [H, H], b16, kind="ExternalInput").ap(),
        "bq_p": nc.dram_tensor("bq_p", [P, HT], f32, kind="ExternalInput").ap(),
        "bv_b": nc.dram_tensor("bv_b", [P, H], f32, kind="ExternalInput").ap(),
        "ones_p": nc.dram_tensor("ones_p", [P, 4], b16, kind="ExternalInput").ap(),
    }
    outs = {
        "self_out": nc.dram_tensor("self_out", [BPC, L, H], f32,
                                   kind="ExternalOutput").ap(),
        "opp_out": nc.dram_tensor("opp_out", [BPC, L, H], f32,
                                  kind="ExternalOutput").ap(),
    }
    with tile.TileContext(nc) as tc:
        with contextlib.ExitStack() as ctx:
            _build_core_kernel(ctx, tc, ins, outs)
    nc.compile()
    _NC_CACHE = nc
    return nc


def kernel(q, opp, Wq, bq, Wk, bk, Wv, bv):
    bf16 = ml_dtypes.bfloat16
    q = np.ascontiguousarray(np.asarray(q, dtype=np.float32)).astype(bf16)
    opp = np.ascontiguousarray(np.asarray(opp, dtype=np.float32)).astype(bf16)
    Wq = np.asarray(Wq, dtype=np.float32)
    Wk = np.asarray(Wk, dtype=np.float32)
    Wv = np.asarray(Wv, dtype=np.float32)
    bq = np.asarray(bq, dtype=np.float32)
    bv = np.asarray(bv, dtype=np.float32)
    # bk is mathematically irrelevant (softmax shift-invariance); unused.

    shared = {
        "WqT": np.ascontiguousarray(Wq.T).astype(bf16),
        "Wk": np.ascontiguousarray(Wk).astype(bf16),
        "WvT": np.ascontiguousarray(Wv.T).astype(bf16),
        "bq_p": np.ascontiguousarray(bq.reshape(HT, P).T),
        "bv_b": np.ascontiguousarray(np.tile(bv, (P, 1))),
        "ones_p": np.ones((P, 4), dtype=bf16),
    }
    in_maps = []
    for c in range(NCORES):
        sl = slice(c * BPC, (c + 1) * BPC)
        in_maps.append({
            "q": np.ascontiguousarray(q[sl]),
            "opp": np.ascontiguousarray(opp[sl]),
            **shared,
        })

    nc = _get_module()
    res = run_bass_kernel_spmd(nc, in_maps, core_ids=list(range(NCORES)))
    self_out = np.concatenate([r["self_out"] for r in res.results], axis=0)
    opp_out = np.concatenate([r["opp_out"] for r in res.results], axis=0)
    return (self_out, opp_out)


# revision 6
# speedup vs baseline: 1.3330x; 1.0525x over previous
"""CrossAttend Trainium2 kernel: 8-way data-parallel over batch.

Full inputs arrive here; we shard batch B=16 across 8 NeuronCores
(2 batch elements per core), replicate the 512x512 projection weights,
run one SPMD Bass/Tile kernel, and concatenate the per-core outputs.

Math notes (validated against the fp32 reference):
  - bk drops out entirely: it shifts every sim row by a constant per q,
    and softmax over k is shift-invariant.
  - qpk := qp @ Wk is shared by both attentions:
        sim  = qpk @ qp.T   (+ per-q const),   sim2 = qpk @ opp.T (+ const)
  - softmax is computed without max-subtraction; the self-attention
    diagonal is zeroed after exp.
  - rowsums come from an extra N=4 matmul against a ones column that
    reuses the PE-resident P^T weights.

Perf structure (from neuron-profile traces):
  - All matmul operands are bf16: same 1 cycle/row streaming as fp32r,
    but stationary operands get FWL (4x faster LDWEIGHTS), which removed
    the +107ns weight-load stall 40% of the fp32r matmuls paid.
  - Inputs are uploaded bf16 and loaded pre-transposed via the DMA xbar
    (dma_start_transpose) - zero PE transposes. All transposes AND the
    weight loads go on one queue (nc.sync) in dependency order: the
    framework serializes DMA-transposes against other DMAs pairwise, and
    each cross-queue hop in that serial chain costs ~3us of semaphore
    latency; same-queue links run back-to-back.
  - Attention is emitted as pexp1, pexp2, out1, out2 so the PE never
    waits on the exp() activations of the phase it just computed.
  - PSUM accumulation stays fp32; exp() output and P are bf16; measured
    end-to-end error of this mix is ~6.4e-3 (budget 2e-2).

On-chip layouts per batch element (all bf16 unless noted):
  qT, qpT, qpkT, oppT : [128, 4, 1024]  (h on partitions)
  v, opp_v            : [128, 8, 512]   (l on partitions)
  PexpT               : [128, 8, 1024]  (k on partitions, q free)
"""

import contextlib
import math

import ml_dtypes
import numpy as np

import concourse.bass as bass
import concourse.mybir as mybir
import concourse.tile as tile
from concourse import bacc
from concourse.bass_utils import run_bass_kernel_spmd

F32 = mybir.dt.float32
B16 = mybir.dt.bfloat16

B = 16
H = 512
L = 1024
P = 128
NCORES = 8
BPC = B // NCORES   # batch elements per core
HT = H // P         # 4 h-tiles
LT = L // P         # 8 l-tiles
QC = L // 512       # 2 q-chunks of 512
SCALE = 1.0 / math.sqrt(H)


def _build_core_kernel(ctx, tc, ins, outs):
    nc = tc.nc
    AF = mybir.ActivationFunctionType

    q_d = ins["q"]          # [BPC, L, H] bf16
    opp_d = ins["opp"]      # [BPC, L, H] bf16
    self_d = outs["self_out"]
    oout_d = outs["opp_out"]

    wpool = ctx.enter_context(tc.tile_pool(name="w", bufs=1))
    inT = ctx.enter_context(tc.tile_pool(name="inT", bufs=4))
    big = ctx.enter_context(tc.tile_pool(name="big", bufs=4))
    vpool = ctx.enter_context(tc.tile_pool(name="v", bufs=4))
    ppool = ctx.enter_context(tc.tile_pool(name="P", bufs=2))
    opool = ctx.enter_context(tc.tile_pool(name="o", bufs=4))
    rpool = ctx.enter_context(tc.tile_pool(name="r", bufs=4))
    ps_mm = ctx.enter_context(tc.tile_pool(name="psmm", bufs=6, space="PSUM"))
    ps_rs = ctx.enter_context(tc.tile_pool(name="psrs", bufs=2, space="PSUM"))

    def load_transposed(src_d, b):
        """DMA xbar transpose [L, H] bf16 -> [128, HT, L] (h on partitions).

        Single queue (nc.sync): concurrent DMA-transposes on two HWDGE
        queues corrupted the loads; same-queue they run back-to-back."""
        xT = inT.tile([P, HT, L], B16, tag="inT")
        for ht in range(HT):
            nc.sync.dma_start_transpose(
                xT[:, ht, :], src_d[b, :, P * ht:P * (ht + 1)])
        return xT

    # --- input chain, dependency-ordered on the sync queue ---
    wq = wpool.tile([P, HT, H], B16, tag="wq")
    nc.sync.dma_start(wq[:], ins["WqT"].rearrange("(ko ki) m -> ki ko m", ki=P))
    bq = wpool.tile([P, HT], F32, tag="bq")
    nc.sync.dma_start(bq[:], ins["bq_p"][:])
    qT = [None, None]
    oppT = [None, None]
    qT[0] = load_transposed(q_d, 0)
    wk = wpool.tile([P, HT, H], B16, tag="wk")
    nc.sync.dma_start(wk[:], ins["Wk"].rearrange("(ko ki) m -> ki ko m", ki=P))
    wv = wpool.tile([P, HT, H], B16, tag="wv")
    nc.sync.dma_start(wv[:], ins["WvT"].rearrange("(ko ki) m -> ki ko m", ki=P))
    bvb = wpool.tile([P, H], F32, tag="bvb")
    nc.sync.dma_start(bvb[:], ins["bv_b"][:])
    ones = wpool.tile([P, 4], B16, tag="ones")
    nc.sync.dma_start(ones[:], ins["ones_p"][:])
    oppT[0] = load_transposed(opp_d, 0)
    qT[1] = load_transposed(q_d, 1)
    oppT[1] = load_transposed(opp_d, 1)

    def proj_T(src_T, w, bias=None):
        """dst[h_out-part, l] = sum_hin w[hin, hout-tile].T @ src_T[hin, l]."""
        dst = big.tile([P, HT, L], B16, tag="big")
        for ht in range(HT):
            for qc in range(QC):
                ps = ps_mm.tile([P, 512], F32, tag="psmm")
                for hc in range(HT):
                    nc.tensor.matmul(
                        ps[:],
                        lhsT=(w[:, hc, P * ht:P * (ht + 1)]),
                        rhs=(src_T[:, hc, 512 * qc:512 * (qc + 1)]),
                        start=(hc == 0),
                        stop=(hc == HT - 1),
                    )
                d = dst[:, ht, 512 * qc:512 * (qc + 1)]
                if bias is not None:
                    nc.scalar.activation(d, ps[:], AF.Identity,
                                         bias=bias[:, ht:ht + 1], scale=1.0)
                else:
                    nc.vector.tensor_copy(d, ps[:])
        return dst

    def proj_nat(src_T, w_rhs, bias_b):
        """dst[l-part, h_out] = src_T[hin, l-tile].T @ w_rhs[hin, hout] + bias."""
        dst = vpool.tile([P, LT, H], B16, tag="v")
        for lt in range(LT):
            ps = ps_mm.tile([P, 512], F32, tag="psmm")
            for hc in range(HT):
                nc.tensor.matmul(
                    ps[:],
                    lhsT=(src_T[:, hc, P * lt:P * (lt + 1)]),
                    rhs=(w_rhs[:, hc, :]),
                    start=(hc == 0),
                    stop=(hc == HT - 1),
                )
            nc.vector.tensor_tensor(dst[:, lt, :], ps[:], bias_b[:],
                                    mybir.AluOpType.add)
        return dst

    def attn_pexp(lhsT_T, qpkT, masked):
        """PexpT[k, q] = exp(scale * lhsT_T.T @ qpkT), diag zeroed if masked."""
        pexp = ppool.tile([P, LT, L], B16, tag="P")
        for ko in range(LT):
            for qc in range(QC):
                ps = ps_mm.tile([P, 512], F32, tag="psmm")
                for hc in range(HT):
                    nc.tensor.matmul(
                        ps[:],
                        lhsT=(lhsT_T[:, hc, P * ko:P * (ko + 1)]),
                        rhs=(qpkT[:, hc, 512 * qc:512 * (qc + 1)]),
                        start=(hc == 0),
                        stop=(hc == HT - 1),
                    )
                d = pexp[:, ko, 512 * qc:512 * (qc + 1)]
                nc.scalar.activation(d, ps[:], AF.Exp, scale=SCALE)
                if masked and qc == ko // (512 // P):
                    m = ko % (512 // P)
                    nc.gpsimd.affine_select(
                        out=d, in_=d,
                        compare_op=mybir.AluOpType.not_equal,
                        fill=0.0, base=P * m,
                        pattern=[[-1, 512]], channel_multiplier=1,
                    )
        return pexp

    def attn_out(pexp, vv, out_d, b):
        """out = (P.T @ v) / rowsum, rowsum via ones matmul on resident P."""
        for qo in range(LT):
            pso = ps_mm.tile([P, 512], F32, tag="psmm")
            psr = ps_rs.tile([P, 4], F32, tag="psrs")
            for ko in range(LT):
                nc.tensor.matmul(
                    pso[:], lhsT=(pexp[:, ko, P * qo:P * (qo + 1)]),
                    rhs=(vv[:, ko, :]),
                    start=(ko == 0), stop=(ko == LT - 1),
                )
                nc.tensor.matmul(
                    psr[:], lhsT=(pexp[:, ko, P * qo:P * (qo + 1)]),
                    rhs=(ones[:, 0:4]),
                    start=(ko == 0), stop=(ko == LT - 1),
                )
            rc = rpool.tile([P, 1], F32, tag="r")
            nc.vector.reciprocal(rc[:], psr[:, 0:1])
            ot = opool.tile([P, 512], F32, tag="o")
            nc.scalar.activation(ot[:], pso[:], AF.Copy, scale=rc[:, 0:1])
            oeng = nc.scalar if qo % 2 == 0 else nc.sync
            oeng.dma_start(out_d[b, P * qo:P * (qo + 1), :], ot[:])

    for b in range(BPC):
        qpT = proj_T(qT[b], wq, bias=bq)
        qpkT = proj_T(qpT, wk)
        vv = proj_nat(qpT, wv, bvb)
        ovv = proj_nat(oppT[b], wv, bvb)
        pexp1 = attn_pexp(qpT, qpkT, masked=True)
        pexp2 = attn_pexp(oppT[b], qpkT, masked=False)
        attn_out(pexp1, vv, self_d, b)
        attn_out(pexp2, ovv, oout_d, b)


_NC_CACHE = None


def _get_module():
    global _NC_CACHE
    if _NC_CACHE is not None:
        return _NC_CACHE
    nc = bacc.Bacc(None, target_bir_lowering=False, debug=False)
    f32 = mybir.dt.float32
    b16 = mybir.dt.bfloat16
    ins = {
        "q": nc.dram_tensor("q", [BPC, L, H], b16, kind="ExternalInput").ap(),
        "opp": nc.dram_tensor("opp", [BPC, L, H], b16, kind="ExternalInput").ap(),
        "WqT": nc.dram_tensor("WqT", [H, H], b16, kind="ExternalInput").ap(),
        "Wk": nc.dram_tensor("Wk", [H, H], b16, kind="ExternalInput").ap(),
        "WvT": nc.dram_tensor("WvT", [H, H], b16, kind="ExternalInput").ap(),
        "bq_p": nc.dram_tensor("bq_p", [P, HT], f32, kind="ExternalInput").ap(),
        "bv_b": nc.dram_tensor("bv_b", [P, H], f32, kind="ExternalInput").ap(),
        "ones_p": nc.dram_tensor("ones_p", [P, 4], b16, kind="ExternalInput").ap(),
    }
    outs = {
        "self_out": nc.dram_tensor("self_out", [BPC, L, H], f32,
                                   kind="ExternalOutput").ap(),
        "opp_out": nc.dram_tensor("opp_out", [BPC, L, H], f32,
                                  kind="ExternalOutput").ap(),
    }
    with tile.TileContext(nc) as tc:
        with contextlib.ExitStack() as ctx:
            _build_core_kernel(ctx, tc, ins, outs)
    nc.compile()
    _NC_CACHE = nc
    return nc


def kernel(q, opp, Wq, bq, Wk, bk, Wv, bv):
    bf16 = ml_dtypes.bfloat16
    q = np.ascontiguousarray(np.asarray(q, dtype=np.float32)).astype(bf16)
    opp = np.ascontiguousarray(np.asarray(opp, dtype=np.float32)).astype(bf16)
    Wq = np.asarray(Wq, dtype=np.float32)
    Wk = np.asarray(Wk, dtype=np.float32)
    Wv = np.asarray(Wv, dtype=np.float32)
    bq = np.asarray(bq, dtype=np.float32)
    bv = np.asarray(bv, dtype=np.float32)
    # bk is mathematically irrelevant (softmax shift-invariance); unused.

    shared = {
        "WqT": np.ascontiguousarray(Wq.T).astype(bf16),
        "Wk": np.ascontiguousarray(Wk).astype(bf16),
        "WvT": np.ascontiguousarray(Wv.T).astype(bf16),
        "bq_p": np.ascontiguousarray(bq.reshape(HT, P).T),
        "bv_b": np.ascontiguousarray(np.tile(bv, (P, 1))),
        "ones_p": np.ones((P, 4), dtype=bf16),
    }
    in_maps = []
    for c in range(NCORES):
        sl = slice(c * BPC, (c + 1) * BPC)
        in_maps.append({
            "q": np.ascontiguousarray(q[sl]),
            "opp": np.ascontiguousarray(opp[sl]),
            **shared,
        })

    nc = _get_module()
    res = run_bass_kernel_spmd(nc, in_maps, core_ids=list(range(NCORES)))
    self_out = np.concatenate([r["self_out"] for r in res.results], axis=0)
    opp_out = np.concatenate([r["opp_out"] for r in res.results], axis=0)
    return (self_out, opp_out)


# revision 9
# speedup vs baseline: 1.3544x; 1.0161x over previous
"""CrossAttend Trainium2 kernel: 8-way data-parallel over batch.

Full inputs arrive here; we shard batch B=16 across 8 NeuronCores
(2 batch elements per core), replicate the 512x512 projection weights,
run one SPMD Bass/Tile kernel, and concatenate the per-core outputs.

Math notes (validated against the fp32 reference):
  - bk drops out entirely: it shifts every sim row by a constant per q,
    and softmax over k is shift-invariant.
  - qpk := qp @ Wk is shared by both attentions:
        sim  = qpk @ qp.T   (+ per-q const),   sim2 = qpk @ opp.T (+ const)
  - softmax is computed without max-subtraction; the self-attention
    diagonal is zeroed after exp.
  - rowsums come from an extra N=4 matmul against a ones column that
    reuses the PE-resident P^T weights.

Perf structure (from neuron-profile traces):
  - All matmul operands are bf16: same 1 cycle/row streaming as fp32r,
    but stationary operands get FWL (4x faster LDWEIGHTS), which removed
    the +107ns weight-load stall 40% of the fp32r matmuls paid.
  - Inputs are uploaded bf16 and loaded pre-transposed via the DMA xbar
    (dma_start_transpose) - zero PE transposes. All transposes AND the
    weight loads go on one queue (nc.sync) in dependency order: the
    framework serializes DMA-transposes against other DMAs pairwise, and
    each cross-queue hop in that serial chain costs ~3us of semaphore
    latency; same-queue links run back-to-back.
  - Attention is emitted as pexp1, pexp2, out1, out2 so the PE never
    waits on the exp() activations of the phase it just computed.
  - PSUM accumulation stays fp32; exp() output and P are bf16; measured
    end-to-end error of this mix is ~6.4e-3 (budget 2e-2).

On-chip layouts per batch element (all bf16 unless noted):
  qT, qpT, qpkT, oppT : [128, 4, 1024]  (h on partitions)
  v, opp_v            : [128, 8, 512]   (l on partitions)
  PexpT               : [128, 8, 1024]  (k on partitions, q free)
"""

import contextlib
import math

import ml_dtypes
import numpy as np

import concourse.bass as bass
import concourse.mybir as mybir
import concourse.tile as tile
from concourse import bacc
from concourse.bass_utils import run_bass_kernel_spmd

F32 = mybir.dt.float32
B16 = mybir.dt.bfloat16

B = 16
H = 512
L = 1024
P = 128
NCORES = 8
BPC = B // NCORES   # batch elements per core
HT = H // P         # 4 h-tiles
LT = L // P         # 8 l-tiles
QC = L // 512       # 2 q-chunks of 512
SCALE = 1.0 / math.sqrt(H)


def _build_core_kernel(ctx, tc, ins, outs):
    nc = tc.nc
    AF = mybir.ActivationFunctionType

    q_d = ins["q"]          # [BPC, L, H] bf16
    opp_d = ins["opp"]      # [BPC, L, H] bf16
    self_d = outs["self_out"]
    oout_d = outs["opp_out"]

    wpool = ctx.enter_context(tc.tile_pool(name="w", bufs=1))
    inT = ctx.enter_context(tc.tile_pool(name="inT", bufs=4))
    big = ctx.enter_context(tc.tile_pool(name="big", bufs=4))
    vpool = ctx.enter_context(tc.tile_pool(name="v", bufs=4))
    ppool = ctx.enter_context(tc.tile_pool(name="P", bufs=2))
    opool = ctx.enter_context(tc.tile_pool(name="o", bufs=4))
    rpool = ctx.enter_context(tc.tile_pool(name="r", bufs=4))
    ps_mm = ctx.enter_context(tc.tile_pool(name="psmm", bufs=6, space="PSUM"))
    ps_rs = ctx.enter_context(tc.tile_pool(name="psrs", bufs=2, space="PSUM"))

    def load_transposed(src_d, b):
        """DMA xbar transpose [L, H] bf16 -> [128, HT, L] (h on partitions).

        Single queue (nc.sync): concurrent DMA-transposes on two HWDGE
        queues corrupted the loads; same-queue they run back-to-back."""
        xT = inT.tile([P, HT, L], B16, tag="inT")
        for ht in range(HT):
            nc.sync.dma_start_transpose(
                xT[:, ht, :], src_d[b, :, P * ht:P * (ht + 1)])
        return xT

    # --- input chain, dependency-ordered on the sync queue ---
    wq = wpool.tile([P, HT, H], B16, tag="wq")
    nc.sync.dma_start(wq[:], ins["WqT"].rearrange("(ko ki) m -> ki ko m", ki=P))
    qT = [None, None]
    oppT = [None, None]
    qT[0] = load_transposed(q_d, 0)
    bq = wpool.tile([P, HT], F32, tag="bq")
    nc.sync.dma_start(bq[:], ins["bq_p"][:])
    wk = wpool.tile([P, HT, H], B16, tag="wk")
    nc.sync.dma_start(wk[:], ins["Wk"].rearrange("(ko ki) m -> ki ko m", ki=P))
    wv = wpool.tile([P, HT, H], B16, tag="wv")
    nc.sync.dma_start(wv[:], ins["WvT"].rearrange("(ko ki) m -> ki ko m", ki=P))
    bvb = wpool.tile([P, H], F32, tag="bvb")
    nc.sync.dma_start(bvb[:], ins["bv_b"][:])
    ones = wpool.tile([P, 4], B16, tag="ones")
    nc.sync.dma_start(ones[:], ins["ones_p"][:])
    oppT[0] = load_transposed(opp_d, 0)
    qT[1] = load_transposed(q_d, 1)
    oppT[1] = load_transposed(opp_d, 1)

    def proj_T(src_T, w, bias=None):
        """dst[h_out-part, l] = sum_hin w[hin, hout-tile].T @ src_T[hin, l].

        hc-major in waves of 4 psum groups: each src slab hc is consumed
        by all live groups as soon as its transpose lands, so the PE ramps
        during the b=0 input transposes instead of waiting for slab 3."""
        dst = big.tile([P, HT, L], B16, tag="big")
        groups = [(ht, qc) for ht in range(HT) for qc in range(QC)]
        for wv_ in range(2):
            wave = groups[4 * wv_:4 * wv_ + 4]
            pss = [ps_mm.tile([P, 512], F32, tag="psmm", name=f"ps{g}")
                   for g in range(4)]
            for hc in range(HT):
                for g, (ht, qc) in enumerate(wave):
                    nc.tensor.matmul(
                        pss[g][:],
                        lhsT=(w[:, hc, P * ht:P * (ht + 1)]),
                        rhs=(src_T[:, hc, 512 * qc:512 * (qc + 1)]),
                        start=(hc == 0),
                        stop=(hc == HT - 1),
                    )
            for g, (ht, qc) in enumerate(wave):
                d = dst[:, ht, 512 * qc:512 * (qc + 1)]
                if bias is not None:
                    nc.scalar.activation(d, pss[g][:], AF.Identity,
                                         bias=bias[:, ht:ht + 1], scale=1.0)
                else:
                    nc.vector.tensor_copy(d, pss[g][:])
        return dst

    def proj_nat(src_T, w_rhs, bias_b):
        """dst[l-part, h_out] = src_T[hin, l-tile].T @ w_rhs[hin, hout] + bias."""
        dst = vpool.tile([P, LT, H], B16, tag="v")
        for lt in range(LT):
            ps = ps_mm.tile([P, 512], F32, tag="psmm")
            for hc in range(HT):
                nc.tensor.matmul(
                    ps[:],
                    lhsT=(src_T[:, hc, P * lt:P * (lt + 1)]),
                    rhs=(w_rhs[:, hc, :]),
                    start=(hc == 0),
                    stop=(hc == HT - 1),
                )
            nc.vector.tensor_tensor(dst[:, lt, :], ps[:], bias_b[:],
                                    mybir.AluOpType.add)
        return dst

    def attn_pexp(lhsT_T, qpkT, masked):
        """PexpT[k, q] = exp(scale * lhsT_T.T @ qpkT), diag zeroed if masked."""
        pexp = ppool.tile([P, LT, L], B16, tag="P")
        for ko in range(LT):
            for qc in range(QC):
                ps = ps_mm.tile([P, 512], F32, tag="psmm")
                for hc in range(HT):
                    nc.tensor.matmul(
                        ps[:],
                        lhsT=(lhsT_T[:, hc, P * ko:P * (ko + 1)]),
                        rhs=(qpkT[:, hc, 512 * qc:512 * (qc + 1)]),
                        start=(hc == 0),
                        stop=(hc == HT - 1),
                    )
                d = pexp[:, ko, 512 * qc:512 * (qc + 1)]
                nc.scalar.activation(d, ps[:], AF.Exp, scale=SCALE)
                if masked and qc == ko // (512 // P):
                    m = ko % (512 // P)
                    nc.gpsimd.affine_select(
                        out=d, in_=d,
                        compare_op=mybir.AluOpType.not_equal,
                        fill=0.0, base=P * m,
                        pattern=[[-1, 512]], channel_multiplier=1,
                    )
        return pexp

    def attn_out(pexp, vv, out_d, b):
        """out = (P.T @ v) / rowsum, rowsum via ones matmul on resident P."""
        for qo in range(LT):
            pso = ps_mm.tile([P, 512], F32, tag="psmm")
            psr = ps_rs.tile([P, 4], F32, tag="psrs")
            for ko in range(LT):
                nc.tensor.matmul(
                    pso[:], lhsT=(pexp[:, ko, P * qo:P * (qo + 1)]),
                    rhs=(vv[:, ko, :]),
                    start=(ko == 0), stop=(ko == LT - 1),
                )
                nc.tensor.matmul(
                    psr[:], lhsT=(pexp[:, ko, P * qo:P * (qo + 1)]),
                    rhs=(ones[:, 0:4]),
                    start=(ko == 0), stop=(ko == LT - 1),
                )
            rc = rpool.tile([P, 1], F32, tag="r")
            nc.vector.reciprocal(rc[:], psr[:, 0:1])
            ot = opool.tile([P, 512], F32, tag="o")
            nc.scalar.activation(ot[:], pso[:], AF.Copy, scale=rc[:, 0:1])
            oeng = nc.scalar if qo % 2 == 0 else nc.sync
            oeng.dma_start(out_d[b, P * qo:P * (qo + 1), :], ot[:])

    for b in range(BPC):
        qpT = proj_T(qT[b], wq, bias=bq)
        qpkT = proj_T(qpT, wk)
        vv = proj_nat(qpT, wv, bvb)
        ovv = proj_nat(oppT[b], wv, bvb)
        pexp1 = attn_pexp(qpT, qpkT, masked=True)
        pexp2 = attn_pexp(oppT[b], qpkT, masked=False)
        attn_out(pexp1, vv, self_d, b)
        attn_out(pexp2, ovv, oout_d, b)


_NC_CACHE = None


def _get_module():
    global _NC_CACHE
    if _NC_CACHE is not None:
        return _NC_CACHE
    nc = bacc.Bacc(None, target_bir_lowering=False, debug=False)
    f32 = mybir.dt.float32
    b16 = mybir.dt.bfloat16
    ins = {
        "q": nc.dram_tensor("q", [BPC, L, H], b16, kind="ExternalInput").ap(),
        "opp": nc.dram_tensor("opp", [BPC, L, H], b16, kind="ExternalInput").ap(),
        "WqT": nc.dram_tensor("WqT", [H, H], b16, kind="ExternalInput").ap(),
        "Wk": nc.dram_tensor("Wk", [H, H], b16, kind="ExternalInput").ap(),
        "WvT": nc.dram_tensor("WvT", [H, H], b16, kind="ExternalInput").ap(),
        "bq_p": nc.dram_tensor("bq_p", [P, HT], f32, kind="ExternalInput").ap(),
        "bv_b": nc.dram_tensor("bv_b", [P, H], f32, kind="ExternalInput").ap(),
        "ones_p": nc.dram_tensor("ones_p", [P, 4], b16, kind="ExternalInput").ap(),
    }
    outs = {
        "self_out": nc.dram_tensor("self_out", [BPC, L, H], f32,
                                   kind="ExternalOutput").ap(),
        "opp_out": nc.dram_tensor("opp_out", [BPC, L, H], f32,
                                  kind="ExternalOutput").ap(),
    }
    with tile.TileContext(nc) as tc:
        with contextlib.ExitStack() as ctx:
            _build_core_kernel(ctx, tc, ins, outs)
    nc.compile()
    _NC_CACHE = nc
    return nc


def kernel(q, opp, Wq, bq, Wk, bk, Wv, bv):
    bf16 = ml_dtypes.bfloat16
    q = np.ascontiguousarray(np.asarray(q, dtype=np.float32)).astype(bf16)
    opp = np.ascontiguousarray(np.asarray(opp, dtype=np.float32)).astype(bf16)
    Wq = np.asarray(Wq, dtype=np.float32)
    Wk = np.asarray(Wk, dtype=np.float32)
    Wv = np.asarray(Wv, dtype=np.float32)
    bq = np.asarray(bq, dtype=np.float32)
    bv = np.asarray(bv, dtype=np.float32)
    # bk is mathematically irrelevant (softmax shift-invariance); unused.

    shared = {
        "WqT": np.ascontiguousarray(Wq.T).astype(bf16),
        "Wk": np.ascontiguousarray(Wk).astype(bf16),
        "WvT": np.ascontiguousarray(Wv.T).astype(bf16),
        "bq_p": np.ascontiguousarray(bq.reshape(HT, P).T),
        "bv_b": np.ascontiguousarray(np.tile(bv, (P, 1))),
        "ones_p": np.ones((P, 4), dtype=bf16),
    }
    in_maps = []
    for c in range(NCORES):
        sl = slice(c * BPC, (c + 1) * BPC)
        in_maps.append({
            "q": np.ascontiguousarray(q[sl]),
            "opp": np.ascontiguousarray(opp[sl]),
            **shared,
        })

    nc = _get_module()
    res = run_bass_kernel_spmd(nc, in_maps, core_ids=list(range(NCORES)))
    self_out = np.concatenate([r["self_out"] for r in res.results], axis=0)
    opp_out = np.concatenate([r["opp_out"] for r in res.results], axis=0)
    return (self_out, opp_out)


# revision 11
# speedup vs baseline: 1.3562x; 1.0013x over previous
"""CrossAttend Trainium2 kernel: 8-way data-parallel over batch.

Full inputs arrive here; we shard batch B=16 across 8 NeuronCores
(2 batch elements per core), replicate the 512x512 projection weights,
run one SPMD Bass/Tile kernel, and concatenate the per-core outputs.

Math notes (validated against the fp32 reference):
  - bk drops out entirely: it shifts every sim row by a constant per q,
    and softmax over k is shift-invariant.
  - qpk := qp @ Wk is shared by both attentions:
        sim  = qpk @ qp.T   (+ per-q const),   sim2 = qpk @ opp.T (+ const)
  - softmax is computed without max-subtraction; the self-attention
    diagonal is zeroed after exp.
  - rowsums come from an extra N=4 matmul against a ones column that
    reuses the PE-resident P^T weights.

Perf structure (from neuron-profile traces):
  - All matmul operands are bf16: same 1 cycle/row streaming as fp32r,
    but stationary operands get FWL (4x faster LDWEIGHTS), which removed
    the +107ns weight-load stall 40% of the fp32r matmuls paid.
  - Inputs are uploaded bf16 and loaded pre-transposed via the DMA xbar
    (dma_start_transpose) - zero PE transposes. All transposes AND the
    weight loads go on one queue (nc.sync) in dependency order: the
    framework serializes DMA-transposes against other DMAs pairwise, and
    each cross-queue hop in that serial chain costs ~3us of semaphore
    latency; same-queue links run back-to-back.
  - Attention is emitted as pexp1, pexp2, out1, out2 so the PE never
    waits on the exp() activations of the phase it just computed.
  - PSUM accumulation stays fp32; exp() output and P are bf16; measured
    end-to-end error of this mix is ~6.4e-3 (budget 2e-2).

On-chip layouts per batch element (all bf16 unless noted):
  qT, qpT, qpkT, oppT : [128, 4, 1024]  (h on partitions)
  v, opp_v            : [128, 8, 512]   (l on partitions)
  PexpT               : [128, 8, 1024]  (k on partitions, q free)
"""

import contextlib
import math

import ml_dtypes
import numpy as np

import concourse.bass as bass
import concourse.mybir as mybir
import concourse.tile as tile
from concourse import bacc
from concourse.bass_utils import run_bass_kernel_spmd

F32 = mybir.dt.float32
B16 = mybir.dt.bfloat16

B = 16
H = 512
L = 1024
P = 128
NCORES = 8
BPC = B // NCORES   # batch elements per core
HT = H // P         # 4 h-tiles
LT = L // P         # 8 l-tiles
QC = L // 512       # 2 q-chunks of 512
SCALE = 1.0 / math.sqrt(H)


def _build_core_kernel(ctx, tc, ins, outs):
    nc = tc.nc
    AF = mybir.ActivationFunctionType

    q_d = ins["q"]          # [BPC, L, H] bf16
    opp_d = ins["opp"]      # [BPC, L, H] bf16
    self_d = outs["self_out"]
    oout_d = outs["opp_out"]

    wpool = ctx.enter_context(tc.tile_pool(name="w", bufs=1))
    inT = ctx.enter_context(tc.tile_pool(name="inT", bufs=4))
    big = ctx.enter_context(tc.tile_pool(name="big", bufs=4))
    vpool = ctx.enter_context(tc.tile_pool(name="v", bufs=4))
    ppool = ctx.enter_context(tc.tile_pool(name="P", bufs=2))
    opool = ctx.enter_context(tc.tile_pool(name="o", bufs=4))
    rpool = ctx.enter_context(tc.tile_pool(name="r", bufs=4))
    ps_mm = ctx.enter_context(tc.tile_pool(name="psmm", bufs=6, space="PSUM"))
    ps_rs = ctx.enter_context(tc.tile_pool(name="psrs", bufs=2, space="PSUM"))

    def load_transposed(src_d, b):
        """DMA xbar transpose [L, H] bf16 -> [128, HT, L] (h on partitions).

        Single queue (nc.sync): concurrent DMA-transposes on two HWDGE
        queues corrupted the loads; same-queue they run back-to-back."""
        xT = inT.tile([P, HT, L], B16, tag="inT")
        for ht in range(HT):
            nc.sync.dma_start_transpose(
                xT[:, ht, :], src_d[b, :, P * ht:P * (ht + 1)])
        return xT

    # --- PE warmup: dummy matmuls on uninitialized scratch while the input
    # chain DMAs run. Keeps the HAM clock-gate at 8/8 and the PE queue hot,
    # so the first real matmul doesn't pay the 1.2GHz ramp + cold-wait lag.
    scratch = wpool.tile([P, 512], B16, tag="scratch")
    nc.vector.memset(scratch[:], 0.0)
    for i in range(20):
        psd = ps_mm.tile([P, 512], F32, tag="psmm", name=f"psd{i}")
        nc.tensor.matmul(psd[:], lhsT=scratch[:, 0:P], rhs=scratch[:],
                         start=True, stop=True)

    # --- input chain, dependency-ordered on the sync queue ---
    wq = wpool.tile([P, HT, H], B16, tag="wq")
    nc.sync.dma_start(wq[:], ins["WqT"].rearrange("(ko ki) m -> ki ko m", ki=P))
    qT = [None, None]
    oppT = [None, None]
    qT[0] = load_transposed(q_d, 0)
    bq = wpool.tile([P, HT], F32, tag="bq")
    nc.sync.dma_start(bq[:], ins["bq_p"][:])
    wk = wpool.tile([P, HT, H], B16, tag="wk")
    nc.sync.dma_start(wk[:], ins["Wk"].rearrange("(ko ki) m -> ki ko m", ki=P))
    wv = wpool.tile([P, HT, H], B16, tag="wv")
    nc.sync.dma_start(wv[:], ins["WvT"].rearrange("(ko ki) m -> ki ko m", ki=P))
    bvb = wpool.tile([P, H], F32, tag="bvb")
    nc.sync.dma_start(bvb[:], ins["bv_b"][:])
    ones = wpool.tile([P, 4], B16, tag="ones")
    nc.sync.dma_start(ones[:], ins["ones_p"][:])
    oppT[0] = load_transposed(opp_d, 0)
    qT[1] = load_transposed(q_d, 1)
    oppT[1] = load_transposed(opp_d, 1)

    def proj_T(src_T, w, bias=None):
        """dst[h_out-part, l] = sum_hin w[hin, hout-tile].T @ src_T[hin, l].

        hc-major in waves of 4 psum groups: each src slab hc is consumed
        by all live groups as soon as its transpose lands, so the PE ramps
        during the b=0 input transposes instead of waiting for slab 3."""
        dst = big.tile([P, HT, L], B16, tag="big")
        groups = [(ht, qc) for ht in range(HT) for qc in range(QC)]
        for wv_ in range(2):
            wave = groups[4 * wv_:4 * wv_ + 4]
            pss = [ps_mm.tile([P, 512], F32, tag="psmm", name=f"ps{g}")
                   for g in range(4)]
            for hc in range(HT):
                for g, (ht, qc) in enumerate(wave):
                    nc.tensor.matmul(
                        pss[g][:],
                        lhsT=(w[:, hc, P * ht:P * (ht + 1)]),
                        rhs=(src_T[:, hc, 512 * qc:512 * (qc + 1)]),
                        start=(hc == 0),
                        stop=(hc == HT - 1),
                    )
            for g, (ht, qc) in enumerate(wave):
                d = dst[:, ht, 512 * qc:512 * (qc + 1)]
                if bias is not None:
                    nc.scalar.activation(d, pss[g][:], AF.Identity,
                                         bias=bias[:, ht:ht + 1], scale=1.0)
                else:
                    nc.vector.tensor_copy(d, pss[g][:])
        return dst

    def proj_nat(src_T, w_rhs, bias_b):
        """dst[l-part, h_out] = src_T[hin, l-tile].T @ w_rhs[hin, hout] + bias."""
        dst = vpool.tile([P, LT, H], B16, tag="v")
        for lt in range(LT):
            ps = ps_mm.tile([P, 512], F32, tag="psmm")
            for hc in range(HT):
                nc.tensor.matmul(
                    ps[:],
                    lhsT=(src_T[:, hc, P * lt:P * (lt + 1)]),
                    rhs=(w_rhs[:, hc, :]),
                    start=(hc == 0),
                    stop=(hc == HT - 1),
                )
            nc.vector.tensor_tensor(dst[:, lt, :], ps[:], bias_b[:],
                                    mybir.AluOpType.add)
        return dst

    def attn_pexp(lhsT_T, qpkT, masked):
        """PexpT[k, q] = exp(scale * lhsT_T.T @ qpkT), diag zeroed if masked."""
        pexp = ppool.tile([P, LT, L], B16, tag="P")
        for ko in range(LT):
            for qc in range(QC):
                ps = ps_mm.tile([P, 512], F32, tag="psmm")
                for hc in range(HT):
                    nc.tensor.matmul(
                        ps[:],
                        lhsT=(lhsT_T[:, hc, P * ko:P * (ko + 1)]),
                        rhs=(qpkT[:, hc, 512 * qc:512 * (qc + 1)]),
                        start=(hc == 0),
                        stop=(hc == HT - 1),
                    )
                d = pexp[:, ko, 512 * qc:512 * (qc + 1)]
                nc.scalar.activation(d, ps[:], AF.Exp, scale=SCALE)
                if masked and qc == ko // (512 // P):
                    m = ko % (512 // P)
                    nc.gpsimd.affine_select(
                        out=d, in_=d,
                        compare_op=mybir.AluOpType.not_equal,
                        fill=0.0, base=P * m,
                        pattern=[[-1, 512]], channel_multiplier=1,
                    )
        return pexp

    def attn_out(pexp, vv, out_d, b):
        """out = (P.T @ v) / rowsum, rowsum via ones matmul on resident P."""
        for qo in range(LT):
            pso = ps_mm.tile([P, 512], F32, tag="psmm")
            psr = ps_rs.tile([P, 4], F32, tag="psrs")
            for ko in range(LT):
                nc.tensor.matmul(
                    pso[:], lhsT=(pexp[:, ko, P * qo:P * (qo + 1)]),
                    rhs=(vv[:, ko, :]),
                    start=(ko == 0), stop=(ko == LT - 1),
                )
                nc.tensor.matmul(
                    psr[:], lhsT=(pexp[:, ko, P * qo:P * (qo + 1)]),
                    rhs=(ones[:, 0:4]),
                    start=(ko == 0), stop=(ko == LT - 1),
                )
            rc = rpool.tile([P, 1], F32, tag="r")
            nc.vector.reciprocal(rc[:], psr[:, 0:1])
            ot = opool.tile([P, 512], F32, tag="o")
            nc.scalar.activation(ot[:], pso[:], AF.Copy, scale=rc[:, 0:1])
            oeng = nc.scalar if qo % 2 == 0 else nc.sync
            oeng.dma_start(out_d[b, P * qo:P * (qo + 1), :], ot[:])

    for b in range(BPC):
        qpT = proj_T(qT[b], wq, bias=bq)
        qpkT = proj_T(qpT, wk)
        vv = proj_nat(qpT, wv, bvb)
        ovv = proj_nat(oppT[b], wv, bvb)
        pexp1 = attn_pexp(qpT, qpkT, masked=True)
        pexp2 = attn_pexp(oppT[b], qpkT, masked=False)
        attn_out(pexp1, vv, self_d, b)
        attn_out(pexp2, ovv, oout_d, b)


_NC_CACHE = None


def _get_module():
    global _NC_CACHE
    if _NC_CACHE is not None:
        return _NC_CACHE
    nc = bacc.Bacc(None, target_bir_lowering=False, debug=False)
    f32 = mybir.dt.float32
    b16 = mybir.dt.bfloat16
    ins = {
        "q": nc.dram_tensor("q", [BPC, L, H], b16, kind="ExternalInput").ap(),
        "opp": nc.dram_tensor("opp", [BPC, L, H], b16, kind="ExternalInput").ap(),
        "WqT": nc.dram_tensor("WqT", [H, H], b16, kind="ExternalInput").ap(),
        "Wk": nc.dram_tensor("Wk", [H, H], b16, kind="ExternalInput").ap(),
        "WvT": nc.dram_tensor("WvT", [H, H], b16, kind="ExternalInput").ap(),
        "bq_p": nc.dram_tensor("bq_p", [P, HT], f32, kind="ExternalInput").ap(),
        "bv_b": nc.dram_tensor("bv_b", [P, H], f32, kind="ExternalInput").ap(),
        "ones_p": nc.dram_tensor("ones_p", [P, 4], b16, kind="ExternalInput").ap(),
    }
    outs = {
        "self_out": nc.dram_tensor("self_out", [BPC, L, H], f32,
                                   kind="ExternalOutput").ap(),
        "opp_out": nc.dram_tensor("opp_out", [BPC, L, H], f32,
                                  kind="ExternalOutput").ap(),
    }
    with tile.TileContext(nc) as tc:
        with contextlib.ExitStack() as ctx:
            _build_core_kernel(ctx, tc, ins, outs)
    nc.compile()
    _NC_CACHE = nc
    return nc


def kernel(q, opp, Wq, bq, Wk, bk, Wv, bv):
    bf16 = ml_dtypes.bfloat16
    q = np.ascontiguousarray(np.asarray(q, dtype=np.float32)).astype(bf16)
    opp = np.ascontiguousarray(np.asarray(opp, dtype=np.float32)).astype(bf16)
    Wq = np.asarray(Wq, dtype=np.float32)
    Wk = np.asarray(Wk, dtype=np.float32)
    Wv = np.asarray(Wv, dtype=np.float32)
    bq = np.asarray(bq, dtype=np.float32)
    bv = np.asarray(bv, dtype=np.float32)
    # bk is mathematically irrelevant (softmax shift-invariance); unused.

    shared = {
        "WqT": np.ascontiguousarray(Wq.T).astype(bf16),
        "Wk": np.ascontiguousarray(Wk).astype(bf16),
        "WvT": np.ascontiguousarray(Wv.T).astype(bf16),
        "bq_p": np.ascontiguousarray(bq.reshape(HT, P).T),
        "bv_b": np.ascontiguousarray(np.tile(bv, (P, 1))),
        "ones_p": np.ones((P, 4), dtype=bf16),
    }
    in_maps = []
    for c in range(NCORES):
        sl = slice(c * BPC, (c + 1) * BPC)
        in_maps.append({
            "q": np.ascontiguousarray(q[sl]),
            "opp": np.ascontiguousarray(opp[sl]),
            **shared,
        })

    nc = _get_module()
    res = run_bass_kernel_spmd(nc, in_maps, core_ids=list(range(NCORES)))
    self_out = np.concatenate([r["self_out"] for r in res.results], axis=0)
    opp_out = np.concatenate([r["opp_out"] for r in res.results], axis=0)
    return (self_out, opp_out)
